# revision 6
# baseline (speedup 1.0000x reference)
"""Trainium2 Bass kernel for nn_MetaBEVWithModalFusion (optimized).

Strategy (8 NeuronCores, SPMD, data-parallel over 512-token query slices):
  - tokens: 4 blocks x 1024 block-tokens = 4096; core c owns block c//2,
    half c%2 (512 q tokens). Feature-major layouts x_T [C, tokens] except
    v / MoE token-major.
  - Attention: logits [k, q] per head on PE; exp on ACT. Softmax
    denominators come FREE from a ones-column folded into v (lhsT M=33
    computes o and s in one accumulation) -- no denominator matmuls.
    Normalization: per-head 1/s rows (DVE reciprocal, bf16), broadcast to
    head strips via a tiny PE ones-matmul, one DVE multiply per strip.
  - wo projections are folded into each attention group's tail (deferred
    emission so the PE queue never stalls on a norm chain); fused
    accumulates in SBUF.
  - Next modality's q/k/v projections are emitted inside the current
    modality's attention (mid-chunk hook) so ACT never idles between mods.
  - MoE: dense experts; gate-weighted sum split across ACT (odd-expert
    scaling via activation-Copy-with-scale), DVE (fused
    scalar_tensor_tensor), Pool (SBUF-only combine adds; GPSIMD cannot
    read PSUM).
  - AllGather split into two 256-token collectives fired as soon as each
    half of the MoE output lands; mo stores go out on the Pool DGE queue.
    Gathered x2 returns token-major and is transposed on the PE via an
    identity matmul (DmaTransposeAnt serializes with collectives, so it is
    avoided); phase-B k2/v1 projections interleave per 4-chunk group and
    the first 16 k-chunks of attention run under the second collective.
  - bf16 matmul operands, fp32 PSUM + softmax statistics.
"""

import math
from contextlib import ExitStack

import ml_dtypes
import numpy as np

import concourse.bass as bass
import concourse.mybir as mybir
import concourse.tile as tile
from concourse.vector_clock import VectorClock, ScopedClock
from concourse.bass_utils import run_bass_kernel_spmd

F32 = mybir.dt.float32
F32R = mybir.dt.float32r
BF = mybir.dt.bfloat16
BF_NP = ml_dtypes.bfloat16
EXP = mybir.ActivationFunctionType.Exp
MULT = mybir.AluOpType.mult
ADD = mybir.AluOpType.add

N_CORES = 8
E = 256
NH = 8
DH = 32
Q = 512  # q tokens per core
DBG = False


def _patched_drain(self, tick_clock, wait_clock):
    # This walrus build cannot encode >1 semaphore wait on the tail Drain
    # (NO_STRUCT); split the final-clock waits across SP NOPs issued before it.
    gc = tick_clock.global_clock
    n = len(gc)
    for p in range(n):
        if gc[p] > 0:
            sub = VectorClock([gc[i] if i == p else 0 for i in range(n)])
            nop = self.nc.sync.nop()
            wait_clock.add_sem_waits(nop.ins, ScopedClock({None: sub}))
    self.nc.sync.drain()
    self.nc.all_engine_barrier()
    popped = self.nc._tile_sem_poison_stack.pop()
    assert popped is self._sem_poison
    self.nc.clear_and_free_semaphores(list(self.sems.allocated().values()))
    self.nc.all_engine_barrier()


tile.TileContext._drain_and_barrier = _patched_drain


def _split_multi_waits(nc):
    """This walrus build encodes at most ONE sem wait per instruction; peel
    excess waits onto same-engine NoOps placed immediately before."""
    for fn in nc.m.functions:
        for bb in fn.blocks:
            new = []
            changed = False
            for inst in bb.instructions:
                si = inst.sync_info
                if si is not None and si.on_wait and len(si.on_wait) > 1:
                    changed = True
                    waits = list(si.on_wait)
                    for w in waits[:-1]:
                        nop = mybir.InstNoOp(
                            name=f"I-wsplit-{nc.next_id()}", ins=[], outs=[]
                        )
                        nop.engine = inst.engine
                        nop.sync_info = mybir.SyncInfo(on_wait=[w], on_update=[])
                        new.append(nop)
                    si.on_wait = [waits[-1]]
                new.append(inst)
            if changed:
                bb.instructions[:] = new


def _proj_fm(nc, pp, spool, name, wT, x_aps, n_tok, bias, ck=512):
    """Feature-major projection: out_T[oc] [128, n_tok] = (W @ x)_chunk + b.

    wT: 2 APs [128(in chunk), 256(out)]; x_aps: 2 APs [128, n_tok];
    bias: 2 APs [128, 1]. Returns two SBUF bf16 tiles [128, n_tok]."""
    outs = []
    nchunks = n_tok // ck
    for oc in range(2):
        o = spool.tile([128, n_tok], BF, tag=f"{name}{oc}", name=f"{name}{oc}")
        outs.append(o)
        for nck in range(nchunks):
            ps = pp.tile([128, 512], F32, tag="P", name="proj_ps", bufs=2)
            for ic in range(2):
                nc.tensor.matmul(
                    ps[:, 0:ck],
                    lhsT=wT[ic][:, 128 * oc : 128 * (oc + 1)],
                    rhs=x_aps[ic][:, ck * nck : ck * (nck + 1)],
                    start=(ic == 0),
                    stop=(ic == 1),
                )
            nc.vector.tensor_scalar_add(
                o[:, ck * nck : ck * (nck + 1)], ps[:, 0:ck], bias[oc][:, 0:1]
            )
    return outs


def _v1_ones(nc, spool, name, n_chunks):
    """Token-major v tile [128, n_chunks*264] with per-head ones columns.

    Chunk c columns [264c, 264c+264): head h strip at 33h..33h+32 (32 v cols
    + 1 ones col)."""
    v1 = spool.tile([128, 264 * n_chunks], BF, tag=name, name=name)
    ones_view = v1[:].rearrange("p (c h w) -> p c h w", c=n_chunks, h=8, w=33)
    nc.vector.memset(ones_view[:, :, :, 32:33], 1.0)
    return v1


def _v1_fill(nc, pp, v1, x_aps, wT, tc_list):
    """Fill v1 chunks tc in tc_list: psum = x_chunk @ Wv, strided-copy into
    the per-head 33-stride layout (on Pool to keep DVE free)."""
    view = v1[:].rearrange("p (c h w) -> p c h w", c=v1.shape[1] // 264, h=8, w=33)
    for ci, xi in tc_list:
        ps = pp.tile([128, 512], F32, tag="P", name="v_ps", bufs=2)
        for ic in range(2):
            nc.tensor.matmul(
                ps[:, 0:256],
                lhsT=x_aps[ic][:, 128 * xi : 128 * (xi + 1)],
                rhs=wT[ic][:],
                start=(ic == 0),
                stop=(ic == 1),
            )
        nc.vector.tensor_copy(
            out=view[:, ci, :, 0:32],
            in_=ps[:, 0:256].rearrange("p (h w) -> p h w", h=8),
        )


def _emit_attn(nc, Lp, op, apool, spool, qT, kT, v1, n_chunks, gtag, ones_f32,
               mid_hook=None, post_g=None, dbg_cb=None, nq=512, qbase=0):
    """Multi-head attention, q=512 feature-major qT, k=n_chunks*128 tokens.

    qT/kT: 2 tiles [128, *] with heads 4g+h at partition strips 32h of group
    g. v1: ones-augmented token-major [128, 264*n_chunks]. Returns [oT0, oT1]
    bf16 [128, 512] (softmax-normalized, feature-major).

    post_g(g, oT) emissions are deferred: g0's flushes after g1's pipeline
    starts; g1's is returned for the caller to flush at the next safe point."""
    oTs = []
    deferred = []
    for g in range(2):
        # two heads per PSUM bank: head pair member j at partition offset 64j
        o_ext = [
            op.tile([128, 512], F32, tag=f"oe{j}", name=f"oe{j}") for j in range(2)
        ]

        def emit_o(As, ci):
            for hp in range(2):
                for hh in range(2):
                    h = 2 * hp + hh
                    hg = 4 * g + h
                    j, r = h // 2, 64 * (h % 2)
                    nc.tensor.matmul(
                        o_ext[j][r : r + 33, 0:nq],
                        lhsT=v1[:, 264 * ci + 33 * hg : 264 * ci + 33 * hg + 33],
                        rhs=As[hp][:, nq * hh : nq * (hh + 1)],
                        tile_position=(0, r),
                        start=(ci == 0),
                        stop=(ci == n_chunks - 1),
                        skip_group_check=True,
                    )

        pend = None  # one-chunk software pipeline: PE never waits on exp
        for ci in range(n_chunks):
            if mid_hook is not None and g == 0 and ci == mid_hook[0]:
                mid_hook[1]()
            if g == 1 and ci == 2 and deferred:
                # flush g0's deferred wo work only after g1's pipeline is
                # rolling, so the PE queue never waits on g0's norm chain
                deferred.pop(0)()
            As = []
            for hp in range(2):
                L = Lp.tile([128, 1024], F32, tag="L", name="L", bufs=2)
                for hh in range(2):
                    h = 2 * hp + hh
                    nc.tensor.matmul(
                        L[:, 512 * hh : 512 * hh + nq],
                        lhsT=kT[g][32 * h : 32 * (h + 1), 128 * ci : 128 * (ci + 1)],
                        rhs=qT[g][32 * h : 32 * (h + 1), qbase : qbase + nq],
                        tile_position=(32 * h, 0),
                        start=True,
                        stop=True,
                    )
                A = apool.tile([128, 1024], BF, tag="A", name="A", bufs=6)
                nc.scalar.activation(
                    A[:, 0 : 2 * nq].rearrange("p (a w) -> p a w", a=2),
                    L[:].rearrange("p (a w) -> p a w", a=2)[:, :, 0:nq],
                    EXP,
                )
                As.append(A)
            if pend is not None:
                emit_o(*pend)
            pend = (As, ci)
        emit_o(*pend)

        # normalization: r = 1/s per head, broadcast on Pool, one DVE mul per
        # head strip
        r4 = [
            spool.tile([1, 512], BF, tag=f"r4_{h}", name=f"r4_{h}")
            for h in range(4)
        ]
        with nc.allow_low_precision(reason="1/s broadcast rows are f32r"):
            for h in range(4):
                j, r = h // 2, 64 * (h % 2)
                nc.vector.reciprocal(r4[h][:, 0:nq], o_ext[j][r + 32 : r + 33, 0:nq])
        # broadcast 1/s to the 32 head partitions on PE (f32r rhs: 1 cyc/row)
        rb = Lp.tile([128, 512], F32, tag="P", name="rb", bufs=2)
        for h in range(4):
            nc.tensor.matmul(
                rb[32 * h : 32 * (h + 1), 0:nq],
                lhsT=ones_f32[0:1, :],
                rhs=r4[h][:, 0:nq],
                tile_position=(0, 32 * h),
                start=True,
                stop=True,
                skip_group_check=True,
            )
        rbs = spool.tile([128, 512], F32, tag="rbs", name="rbs")
        nc.vector.tensor_copy(out=rbs[:, 0:nq], in_=rb[:, 0:nq])
        oT = spool.tile([128, 512], BF, tag=f"oT{gtag}{g}q{qbase}", name=f"oT{g}")
        for h in range(4):
            j, r = h // 2, 64 * (h % 2)
            nc.vector.tensor_mul(
                oT[32 * h : 32 * (h + 1), 0:nq], o_ext[j][r : r + 32, 0:nq],
                rbs[32 * h : 32 * (h + 1), 0:nq],
            )
        oTs.append(oT)
        if dbg_cb is not None:
            dbg_cb(g, oT, o_ext)
        if post_g is not None:
            deferred.append(lambda g=g, oT=oT: post_g(g, oT, qbase, nq))
    return oTs, deferred


def build_nc(split_waits=True):
    nc = bass.Bass(num_devices=N_CORES)
    dbg_outs = {}
    def dbg(name, shape, dt=F32):
        if DBG:
            dbg_outs[name] = nc.declare_dram_parameter(
                f"dbg_{name}", list(shape), dt, isOutput=True)
        return dbg_outs.get(name)

    # ---- I/O declarations ----
    def din(name, shape, dt=BF):
        return nc.declare_dram_parameter(name, list(shape), dt, isOutput=False)

    xq = din("xq", (E, Q))
    xd = din("xd", (E, 1024))
    xl = din("xl", (E, 1024))
    xe = din("xe", (E, 1024))
    wmod = {m: din(f"w_{m}", (E, 1024)) for m in "dle"}
    wB = din("wB", (E, 1032))
    wexp = din("wexp", (E, 2048))
    bA = din("bA", (E, 10), F32)
    brow = din("brow", (1, 2056))
    identp = din("ident", (128, 128))
    OUT = nc.declare_dram_parameter("out", [E, Q], F32, isOutput=True)

    with tile.TileContext(nc) as tc, ExitStack() as top:
        wpool = top.enter_context(tc.tile_pool(name="w", bufs=1))
        xpool = top.enter_context(tc.tile_pool(name="x", bufs=1))
        spool = top.enter_context(tc.tile_pool(name="s", bufs=1))
        apool = top.enter_context(tc.tile_pool(name="a", bufs=2))
        dram = top.enter_context(tc.tile_pool(name="dram", bufs=1, space="DRAM"))

        ones_row = wpool.tile([1, 128], BF, tag="ones_row", name="ones_row")
        nc.vector.memset(ones_row[:], 1.0)
        ones_f32 = wpool.tile([1, 32], BF, tag="ones_f32", name="ones_f32")
        nc.vector.memset(ones_f32[:], 1.0)
        # identity for PE-based [128,128] transposes (out = lhsT.T @ I)
        ident = wpool.tile([128, 128], BF, tag="ident", name="ident")
        nc.sync.dma_start(out=ident[:], in_=identp[:])

        # ---- input DMAs (merged; order = need order) ----
        _dma_engs = [nc.sync, nc.scalar, nc.gpsimd]
        _dma_rr = [0]

        def load_pair(pool, name, param, dt, width):
            ts = []
            for ic in range(2):
                t = pool.tile([128, width], dt, tag=f"{name}{ic}", name=f"{name}{ic}")
                eng = _dma_engs[_dma_rr[0] % len(_dma_engs)]
                _dma_rr[0] += 1
                eng.dma_start(out=t[:], in_=param[128 * ic : 128 * (ic + 1), :])
                ts.append(t)
            return ts

        xq_t = load_pair(xpool, "xq", xq, BF, Q)
        w_t = {"d": load_pair(wpool, "wd", wmod["d"], BF, 1024)}
        xd_t = load_pair(xpool, "xd", xd, BF, 1024)
        ba_t = load_pair(wpool, "ba", bA, F32, 10)
        xl_t = load_pair(xpool, "xl", xl, BF, 1024)
        w_t["l"] = load_pair(wpool, "wl", wmod["l"], BF, 1024)
        xe_t = load_pair(xpool, "xe", xe, BF, 1024)
        w_t["e"] = load_pair(wpool, "we", wmod["e"], BF, 1024)
        wb_t = load_pair(wpool, "wB", wB, BF, 1032)
        wexp_t = load_pair(wpool, "wexp", wexp, BF, 2048)
        br_t = wpool.tile([1, 2056], BF, tag="br", name="br")
        nc.sync.dma_start(out=br_t[:], in_=brow[:])

        x_aps = {
            "d": [t[:] for t in xd_t],
            "l": [t[:] for t in xl_t],
            "e": [t[:] for t in xe_t],
        }
        xq_aps = [t[:] for t in xq_t]

        def wslice(m, j):  # j: 0=wq 1=wk 2=wv 3=wo
            return [w_t[m][ic][:, 256 * j : 256 * (j + 1)] for ic in range(2)]

        def wbslice(j):  # 0=wq_f 1=wk_f 2=wv_f 3=wo_f
            return [wb_t[ic][:, 256 * j : 256 * (j + 1)] for ic in range(2)]

        wg_aps = [wb_t[ic][:, 1024:1032] for ic in range(2)]

        def bslice(j):
            return [ba_t[ic][:, j : j + 1] for ic in range(2)]

        ag_in = [
            dram.tile([256, E], BF, tag=f"ag_in{h}", name=f"ag_in{h}")
            for h in range(2)
        ]
        ag_out = [
            dram.tile(
                [N_CORES * 256, E], BF, addr_space="Shared",
                tag=f"ag_out{h}", name=f"ag_out{h}",
            )
            for h in range(2)
        ]

        # ============ Phase A: per-mod projection + attention ============
        # wo is folded into each attention group's tail: fused accumulates in
        # SBUF so the MoE can start right after the last norm
        facc = [
            spool.tile([128, 512], F32, tag=f"facc{oc}", name=f"facc{oc}")
            for oc in range(2)
        ]
        fused_sb = [
            spool.tile([128, 512], BF, tag=f"fused{oc}", name=f"fused{oc}")
            for oc in range(2)
        ]

        moeT = [
            spool.tile([128, Q], BF, tag=f"moeT{fc}", name=f"moeT{fc}")
            for fc in range(2)
        ]
        x2Tt = {
            h: [
                spool.tile([128, 2048], BF, tag=f"x2T{h}{ic}", name=f"x2T{h}{ic}")
                for ic in range(2)
            ]
            for h in range(2)
        }
        xtm = [
            spool.tile([128, 4096], BF, tag=f"xtm{h}", name=f"xtm{h}")
            for h in range(2)
        ]
        COPY = mybir.ActivationFunctionType.Copy

        def fire_collective(h):
            nc.gpsimd.collective_compute(
                "AllGather",
                mybir.AluOpType.bypass,
                replica_groups=[list(range(N_CORES))],
                ins=[ag_in[h][:].opt()],
                outs=[ag_out[h][:].opt()],
            )

        pending_wo = []
        with tc.tile_pool(name="Ap", bufs=1, space="PSUM") as Ap:
            projs = {}

            def emit_proj(m, mi):
                qT = _proj_fm(
                    nc, Ap, spool, f"qT_{m}", wslice(m, 0), xq_aps, Q, bslice(2 * mi)
                )
                kT = _proj_fm(
                    nc, Ap, spool, f"kT_{m}", wslice(m, 1), x_aps[m], 1024,
                    bslice(2 * mi + 1),
                )
                v1 = _v1_ones(nc, spool, f"v1_{m}", 8)
                _v1_fill(nc, Ap, v1, x_aps[m], wslice(m, 2), [(c, c) for c in range(8)])
                projs[m] = (qT, kT, v1)

            def emit_moe(tcns):
                gns = {}
                for tcn in tcns:
                    gps = Ap.tile([128, 512], F32, tag="P", name="g_ps", bufs=2)
                    for ic in range(2):
                        nc.tensor.matmul(
                            gps[:, 0:NH],
                            lhsT=fused_sb[ic][:, 128 * tcn : 128 * (tcn + 1)],
                            rhs=wg_aps[ic],
                            start=(ic == 0),
                            stop=False,
                        )
                    nc.tensor.matmul(
                        gps[:, 0:NH], lhsT=ones_row[:], rhs=br_t[0:1, 0:NH],
                        start=False, stop=True,
                    )
                    eg = apool.tile([128, NH], F32, tag="eg", name="eg", bufs=4)
                    nc.scalar.activation(eg[:], gps[:, 0:NH], EXP)
                    sg = apool.tile([128, 1], F32, tag="sg", name="sg", bufs=4)
                    nc.vector.tensor_reduce(
                        sg[:], eg[:], axis=mybir.AxisListType.X, op=ADD
                    )
                    rg = apool.tile([128, 1], F32, tag="rg", name="rg", bufs=4)
                    nc.vector.reciprocal(rg[:], sg[:])
                    gn = spool.tile([128, NH], F32, tag=f"gn{tcn}", name=f"gn{tcn}")
                    nc.vector.tensor_scalar_mul(gn[:], eg[:], rg[:, 0:1])
                    gns[tcn] = gn

                for tcn in tcns:
                    gn = gns[tcn]
                    ms = [
                        apool.tile([128, E], F32, tag=f"m{e}", name=f"m{e}", bufs=1)
                        for e in range(4)
                    ]
                    cs = [
                        apool.tile([128, E], F32, tag=f"c{e}", name=f"c{e}", bufs=1)
                        for e in range(4)
                    ]
                    d0 = apool.tile([128, E], F32, tag="d0", name="d0", bufs=1)
                    d1 = apool.tile([128, E], F32, tag="d1", name="d1", bufs=1)
                    mo = apool.tile([128, E], BF, tag=f"mo{tcn}", name=f"mo{tcn}")
                    for e in (1, 0, 3, 2, 5, 4, 7, 6):
                        yps = Ap.tile([128, 512], F32, tag="P", name="y_ps", bufs=2)
                        for ic in range(2):
                            nc.tensor.matmul(
                                yps[:, 0:E],
                                lhsT=fused_sb[ic][:, 128 * tcn : 128 * (tcn + 1)],
                                rhs=wexp_t[ic][:, 256 * e : 256 * (e + 1)],
                                start=(ic == 0),
                                stop=False,
                            )
                        nc.tensor.matmul(
                            yps[:, 0:E],
                            lhsT=ones_row[:],
                            rhs=br_t[0:1, 8 + 256 * e : 8 + 256 * (e + 1)],
                            start=False,
                            stop=True,
                        )
                        if e % 2 == 1:
                            nc.scalar.activation(
                                ms[e // 2][:], yps[:, 0:E], COPY,
                                scale=gn[:, e : e + 1],
                            )
                        else:
                            nc.vector.scalar_tensor_tensor(
                                out=cs[e // 2][:],
                                in0=yps[:, 0:E],
                                scalar=gn[:, e : e + 1],
                                in1=ms[e // 2][:],
                                op0=MULT,
                                op1=ADD,
                            )
                    nc.gpsimd.tensor_add(d0[:], cs[0][:], cs[1][:])
                    nc.gpsimd.tensor_add(d1[:], cs[2][:], cs[3][:])
                    nc.gpsimd.tensor_add(mo[:], d0[:], d1[:])
                    nc.gpsimd.dma_start(
                        out=ag_in[tcn // 2][128 * (tcn % 2) : 128 * (tcn % 2 + 1), :],
                        in_=mo[:],
                    )
                    for fc in range(2):
                        tp = Ap.tile([128, 512], F32, tag="P", name="tp", bufs=2)
                        nc.tensor.matmul(
                            tp[:, 0:128],
                            lhsT=mo[:, 128 * fc : 128 * (fc + 1)],
                            rhs=ident[:],
                            start=True,
                            stop=True,
                        )
                        nc.vector.tensor_copy(
                            out=moeT[fc][:, 128 * tcn : 128 * (tcn + 1)],
                            in_=tp[:, 0:128],
                        )

            emit_proj("d", 0)
            for mi, m in enumerate("dl"):
                qT, kT, v1 = projs[m]

                def hook_fn(mi=mi):
                    emit_proj("dle"[mi + 1], mi + 1)
                    for fl in pending_wo:
                        fl()
                    del pending_wo[:]

                def post_g(g, oT, qbase, nq, m=m, mi=mi):
                    sl = slice(qbase, qbase + nq)
                    for oc in range(2):
                        ps = Ap.tile([128, 512], F32, tag="P", name="wo_ps", bufs=2)
                        nc.tensor.matmul(
                            ps[:, 0:nq],
                            lhsT=wslice(m, 3)[g][:, 128 * oc : 128 * (oc + 1)],
                            rhs=oT[:, 0:nq],
                            start=True,
                            stop=True,
                        )
                        if mi == 0 and g == 0:
                            nc.vector.tensor_scalar_add(
                                facc[oc][:, sl], ps[:, 0:nq], bslice(6)[oc][:, 0:1]
                            )
                        else:
                            nc.vector.tensor_add(
                                facc[oc][:, sl], ps[:, 0:nq], facc[oc][:, sl]
                            )

                _, new_pending = _emit_attn(
                    nc, Ap, Ap, apool, spool, qT, kT, v1, 8, gtag=m,
                    ones_f32=ones_f32, post_g=post_g, mid_hook=(5, hook_fn),
                )
                pending_wo.extend(new_pending)

            qT, kT, v1 = projs["e"]

            def post_g_e(g, oT, qbase, nq):
                sl = slice(qbase, qbase + nq)
                for oc in range(2):
                    ps = Ap.tile([128, 512], F32, tag="P", name="wo_ps", bufs=2)
                    nc.tensor.matmul(
                        ps[:, 0:nq],
                        lhsT=wslice("e", 3)[g][:, 128 * oc : 128 * (oc + 1)],
                        rhs=oT[:, 0:nq],
                        start=True,
                        stop=True,
                    )
                    if g == 1:
                        nc.vector.tensor_add(
                            fused_sb[oc][:, sl], ps[:, 0:nq], facc[oc][:, sl]
                        )
                    else:
                        nc.vector.tensor_add(
                            facc[oc][:, sl], ps[:, 0:nq], facc[oc][:, sl]
                        )

            def hook_h0():
                for fl in pending_wo:
                    fl()
                del pending_wo[:]

            _, pend0 = _emit_attn(
                nc, Ap, Ap, apool, spool, qT, kT, v1, 8, gtag="e",
                ones_f32=ones_f32, post_g=post_g_e, mid_hook=(5, hook_h0),
                nq=256, qbase=0,
            )

            def hook_h1():
                for fl in pend0:
                    fl()
                emit_moe((0, 1))

            _, pend1 = _emit_attn(
                nc, Ap, Ap, apool, spool, qT, kT, v1, 8, gtag="e",
                ones_f32=ones_f32, post_g=post_g_e, mid_hook=(3, hook_h1),
                nq=256, qbase=256,
            )
            fire_collective(0)
            for fl in pend1:
                fl()
            emit_moe((2, 3))
            fire_collective(1)

        with tc.tile_pool(name="ppq2", bufs=1, space="PSUM") as ppq2:
            q2 = _proj_fm(
                nc, ppq2, spool, "q2T",
                wbslice(0), [t[:] for t in moeT], Q, [bslice(7)[oc] for oc in range(2)],
            )

        # ================= Phase B =================
        with tc.tile_pool(name="Lp2", bufs=1, space="PSUM") as Lp2:
            op2 = Lp2
            k2 = [
                spool.tile([128, 4096], BF, tag=f"k2T{oc}", name=f"k2T{oc}")
                for oc in range(2)
            ]
            v1B = _v1_ones(nc, spool, "v1B", 32)
            x2T = x2Tt

            def emit_half(h):
                # gathered tokens arrive token-major; transpose on PE via the
                # identity matmul (no DmaTransposeAnt -> no collective
                # serialization). Interleave per 4-token-chunk group so k2/v1
                # start as soon as their columns exist.
                xv = xtm[h][:].rearrange("p (j f) -> p j f", j=16)
                gv = ag_out[h][:].rearrange("(j p) f -> p j f", p=128)
                nc.sync.dma_start(out=xv[:, 0:4, :], in_=gv[:, 0:4, :])
                nc.sync.dma_start(out=xv[:, 4:16, :], in_=gv[:, 4:16, :])
                for grp in range(4):
                    for tc in range(4 * grp, 4 * grp + 4):
                        for fc in range(2):
                            ps = Lp2.tile([128, 512], F32, tag="P", name="t_ps", bufs=2)
                            nc.tensor.matmul(
                                ps[:, 0:128],
                                lhsT=xtm[h][:, 256 * tc + 128 * fc : 256 * tc + 128 * (fc + 1)],
                                rhs=ident[:],
                                start=True,
                                stop=True,
                            )
                            if h == 0 and (tc + fc) % 2 == 0:
                                nc.scalar.activation(
                                    x2T[h][fc][:, 128 * tc : 128 * (tc + 1)],
                                    ps[:, 0:128],
                                    COPY,
                                )
                            else:
                                nc.vector.tensor_copy(
                                    out=x2T[h][fc][:, 128 * tc : 128 * (tc + 1)],
                                    in_=ps[:, 0:128],
                                )
                    nck = grp
                    for oc in range(2):
                        ps = Lp2.tile([128, 512], F32, tag="P", name="k2_ps", bufs=2)
                        for ic in range(2):
                            nc.tensor.matmul(
                                ps[:],
                                lhsT=wbslice(1)[ic][:, 128 * oc : 128 * (oc + 1)],
                                rhs=x2T[h][ic][:, 512 * nck : 512 * (nck + 1)],
                                start=(ic == 0),
                                stop=(ic == 1),
                            )
                        nc.vector.tensor_scalar_add(
                            k2[oc][:, 2048 * h + 512 * nck : 2048 * h + 512 * (nck + 1)],
                            ps[:],
                            bslice(8)[oc][:, 0:1],
                        )
                    _v1_fill(
                        nc, Lp2, v1B, [t[:] for t in x2T[h]], wbslice(2),
                        [(16 * h + i, i) for i in range(4 * grp, 4 * grp + 4)],
                    )

            emit_half(0)
            t = dbg("q2", (128, 512), BF)
            if t is not None:
                nc.sync.dma_start(out=t[:], in_=q2[0][:])
            oaccB = [
                spool.tile([128, 512], F32, tag=f"oaccB{oc}", name=f"oaccB{oc}")
                for oc in range(2)
            ]

            def post_gB(g, oT, qbase, nq):
                for oc in range(2):
                    ps = Lp2.tile([128, 512], F32, tag="P", name="wof_ps", bufs=2)
                    nc.tensor.matmul(
                        ps[:],
                        lhsT=wbslice(3)[g][:, 128 * oc : 128 * (oc + 1)],
                        rhs=oT[:],
                        start=True,
                        stop=True,
                    )
                    if g == 0:
                        nc.vector.tensor_scalar_add(
                            oaccB[oc][:], ps[:], bslice(9)[oc][:, 0:1]
                        )
                    else:
                        osb = apool.tile([128, 512], F32, tag="osb", name="osb")
                        nc.vector.tensor_add(osb[:], ps[:], oaccB[oc][:])
                        nc.sync.dma_start(
                            out=OUT[128 * oc : 128 * (oc + 1), :], in_=osb[:]
                        )

            def dbg_cbf(g, oT, o_ext):
                t = dbg(f"oTf{g}", (128, 512), BF)
                if t is not None:
                    nc.sync.dma_start(out=t[:], in_=oT[:])
                if g == 0:
                    t2 = dbg("oe0raw", (128, 512), F32)
                    if t2 is not None:
                        osb0 = spool.tile([128, 512], F32, tag="dbgoe", name="dbgoe")
                        nc.vector.tensor_copy(out=osb0[:], in_=o_ext[0][:])
                        nc.sync.dma_start(out=t2[:], in_=osb0[:])
            oTf, pend_f = _emit_attn(
                nc, Lp2, op2, apool, spool, q2, k2, v1B, 32, gtag="f",
                ones_f32=ones_f32, mid_hook=(16, lambda: emit_half(1)),
                post_g=post_gB, dbg_cb=dbg_cbf if DBG else None,
            )
            for fl in pend_f:
                fl()
            t = dbg("k2", (128, 4096), BF)
            if t is not None:
                nc.sync.dma_start(out=t[:], in_=k2[0][:])
            t = dbg("v1B", (128, 1024), BF)
            if t is not None:
                nc.sync.dma_start(out=t[:], in_=v1B[:, 0:1024])
            t = dbg("xtm0", (128, 1024), BF)
            if t is not None:
                nc.sync.dma_start(out=t[:], in_=xtm[0][:, 0:1024])
            t = dbg("x2T00", (128, 1024), BF)
            if t is not None:
                nc.sync.dma_start(out=t[:], in_=x2T[0][0][:, 0:1024])

    if split_waits:
        _split_multi_waits(nc)
    return nc


# ------------------------------------------------------------------
# Host side
# ------------------------------------------------------------------

def _prep_maps(inputs):
    f32 = lambda a: np.ascontiguousarray(np.asarray(a, dtype=np.float32))
    bf = lambda a: np.ascontiguousarray(np.asarray(a).astype(BF_NP))
    s32 = math.sqrt(DH)

    imgs = {m: f32(inputs[n])[0] for m, n in (("d", "B_depth"), ("l", "B_lidar"), ("e", "B_event"))}

    shared = {}
    bA = np.zeros((E, 10), np.float32)
    for mi, m in enumerate("dle"):
        Wi, bi = f32(inputs[f"Wi_{m}"]), f32(inputs[f"bi_{m}"])
        Wo = f32(inputs[f"Wo_{m}"])
        shared[f"w_{m}"] = bf(
            np.concatenate(
                [(Wi[:E] / (3.0 * s32)).T, Wi[E : 2 * E].T, Wi[2 * E :].T, Wo.T],
                axis=1,
            )
        )
        bA[:, 2 * mi] = bi[:E] / s32
        bA[:, 2 * mi + 1] = bi[E : 2 * E]
    bo_sum = np.zeros(E, np.float32)
    for m in "dle":
        Wi, bi = f32(inputs[f"Wi_{m}"]), f32(inputs[f"bi_{m}"])
        Wo, bo = f32(inputs[f"Wo_{m}"]), f32(inputs[f"bo_{m}"])
        bo_sum += bo + Wo @ bi[2 * E :]
    bA[:, 6] = bo_sum

    Wi, bi = f32(inputs["Wi_m"]), f32(inputs["bi_m"])
    Wo, bo = f32(inputs["Wo_m"]), f32(inputs["bo_m"])
    shared["wB"] = bf(
        np.concatenate(
            [(Wi[:E] / s32).T, Wi[E : 2 * E].T, Wi[2 * E :].T, Wo.T,
             f32(inputs["Wg"]).T],
            axis=1,
        )
    )
    bA[:, 7] = bi[:E] / s32
    bA[:, 8] = bi[E : 2 * E]
    bA[:, 9] = bo + Wo @ bi[2 * E :]
    shared["bA"] = bA

    We = f32(inputs["We"])
    shared["wexp"] = bf(np.concatenate([We[e].T for e in range(NH)], axis=1))
    shared["brow"] = bf(
        np.concatenate([f32(inputs["bg"]), f32(inputs["be"]).reshape(-1)])
    ).reshape(1, 2056)
    shared["ident"] = bf(np.eye(128, dtype=np.float32))

    in_maps = []
    for c in range(N_CORES):
        b, h2 = c // 2, c % 2
        hb, wb = b // 2, b % 2
        blk = {
            m: imgs[m][:, 32 * hb : 32 * (hb + 1), 32 * wb : 32 * (wb + 1)].reshape(E, 1024)
            for m in "dle"
        }
        xsum = blk["d"] + blk["l"] + blk["e"]
        im = dict(shared)
        im["xq"] = bf(xsum[:, Q * h2 : Q * (h2 + 1)])
        im["xd"] = bf(blk["d"])
        im["xl"] = bf(blk["l"])
        im["xe"] = bf(blk["e"])
        in_maps.append(im)
    return in_maps


_NC_CACHE = {}


def _get_nc():
    if "nc" not in _NC_CACHE:
        _NC_CACHE["nc"] = build_nc()
    return _NC_CACHE["nc"]


def _assemble(results):
    out = np.zeros((1, E, 64, 64), np.float32)
    for c in range(N_CORES):
        b, h2 = c // 2, c % 2
        hb, wb = b // 2, b % 2
        o = results[c]["out"].reshape(E, 16, 32)
        out[0, :, 32 * hb + 16 * h2 : 32 * hb + 16 * (h2 + 1), 32 * wb : 32 * (wb + 1)] = o
    return out


def kernel(**inputs):
    nc = _get_nc()
    in_maps = _prep_maps(inputs)
    res = run_bass_kernel_spmd(nc, in_maps, core_ids=list(range(N_CORES)))
    return _assemble(res.results)
